# revision 6
# baseline (speedup 1.0000x reference)
"""Cross-attention kernel for Trainium2, data-parallel over batch on 8 NeuronCores.

Per core (local batch BL=2):
  qT[a,lq] = Wq^T @ Xq^T      (fp32r matmuls, 11-bit RNE operands)
  kT[a,lk] = Wk^T @ Xkv^T     (fp32r)
  v[lk,a]  = Xkv @ Wv         (fp32r)
  sT[lk,lq] = kT^T... scores transposed: lhsT=kT, rhs=qT  (fp32r)
  eT = exp(sT - G)            (ACT, bf16 out; G fixed stabilizer)
  D[q] = sum_lk eT            (DVE tree-add + matmul rider)
  CT[a,lq] = v^T @ eT-ish     (bf16)
  out[lq,e] = (CT^T @ Wo) * (1/D) + (bv @ Wo + bo)   (bf16 matmul, fp32 epilogue)

All input transposes (Xq^T, Xkv^T) via PE transpose-mode on 128x128 blocks.
"""
import numpy as np

import concourse.bass as bass
import concourse.bacc as bacc
import concourse.tile as tile
from concourse import mybir
from concourse.bass_utils import run_bass_kernel_spmd

B, LQ, LK, E, F, A = 16, 1024, 2048, 512, 256, 512
NCORES = 8
BL = B // NCORES
G = 100.0

f32 = mybir.dt.float32
f32r = mybir.dt.float32r
bf16 = mybir.dt.bfloat16

QT = LQ // 128   # 8  q 128-tiles
KT = LK // 128   # 16 lk 128-tiles
ET = E // 128    # 4  e 128-tiles
FT = F // 128    # 2  f 128-tiles
AT = A // 128    # 4  a 128-tiles
QC = LQ // 512   # 2  q 512-chunks
KC = LK // 512   # 4  lk 512-chunks


def _body(tc):
    nc = tc.nc
    lstm = nc.dram_tensor("lstm_embeddings", [BL, LQ, E], f32, kind="ExternalInput").ap()
    flow = nc.dram_tensor("optical_flow_features", [BL, LK, F], f32, kind="ExternalInput").ap()
    Wq_d = nc.dram_tensor("Wq", [E, A], f32, kind="ExternalInput").ap()
    bq_d = nc.dram_tensor("bq", [A], f32, kind="ExternalInput").ap()
    Wk_d = nc.dram_tensor("Wk", [F, A], f32, kind="ExternalInput").ap()
    bk_d = nc.dram_tensor("bk", [A], f32, kind="ExternalInput").ap()
    Wv_d = nc.dram_tensor("Wv", [F, A], f32, kind="ExternalInput").ap()
    bv_d = nc.dram_tensor("bv", [A], f32, kind="ExternalInput").ap()
    Wo_d = nc.dram_tensor("Wo", [A, E], f32, kind="ExternalInput").ap()
    bo_d = nc.dram_tensor("bo", [E], f32, kind="ExternalInput").ap()
    id_d = nc.dram_tensor("ident_", [128, 128], f32, kind="ExternalInput").ap()
    ones_d = nc.dram_tensor("ones_", [128, 1], f32, kind="ExternalInput").ap()
    out_d = nc.dram_tensor("out", [BL, LQ, E], f32, kind="ExternalOutput").ap()

    from contextlib import ExitStack
    with ExitStack() as ctx:
        wp = ctx.enter_context(tc.tile_pool(name="wp", bufs=1))
        stage = ctx.enter_context(tc.tile_pool(name="stage", bufs=1))
        big = ctx.enter_context(tc.tile_pool(name="big", bufs=1))
        small = ctx.enter_context(tc.tile_pool(name="small", bufs=2))
        pt = ctx.enter_context(tc.tile_pool(name="pt", bufs=2, space="PSUM"))
        pm = ctx.enter_context(tc.tile_pool(name="pm", bufs=4, space="PSUM"))
        pdp = ctx.enter_context(tc.tile_pool(name="pdp", bufs=2, space="PSUM"))

        # ---- persistent weights / constants (DMA-cast to fp32r on load) ----
        Wq_r = wp.tile([128, ET, A], f32r)
        nc.gpsimd.dma_start(Wq_r[:], Wq_d.rearrange("(t p) a -> p t a", p=128))
        Wk_r = wp.tile([128, FT, A], f32r)
        nc.gpsimd.dma_start(Wk_r[:], Wk_d.rearrange("(t p) a -> p t a", p=128))
        Wv_r = wp.tile([128, FT, A], f32r)
        nc.gpsimd.dma_start(Wv_r[:], Wv_d.rearrange("(t p) a -> p t a", p=128))
        Wo_bf = wp.tile([128, AT, E], bf16)
        nc.gpsimd.dma_start(Wo_bf[:], Wo_d.rearrange("(t p) e -> p t e", p=128))
        ident_r = wp.tile([128, 128], f32r)
        nc.gpsimd.dma_start(ident_r[:], id_d[:])
        ones_r = wp.tile([128, 1], f32r)
        nc.gpsimd.dma_start(ones_r[:], ones_d[:])

        bq_sb = wp.tile([128, AT], f32)
        nc.sync.dma_start(bq_sb[:], bq_d.rearrange("(t p) -> p t", p=128))
        bk_sb = wp.tile([128, AT], f32)
        nc.sync.dma_start(bk_sb[:], bk_d.rearrange("(t p) -> p t", p=128))
        bv_sb = wp.tile([128, AT], f32)
        nc.sync.dma_start(bv_sb[:], bv_d.rearrange("(t p) -> p t", p=128))
        # bo broadcast across partitions straight from DRAM (DRE replication)
        boB = wp.tile([128, E], f32)
        bo_bcast_ap = bass.AP(tensor=bo_d.tensor, offset=bo_d.offset,
                              ap=[[0, 128]] + list(bo_d.ap))
        nc.gpsimd.dma_start(boB[:], bo_bcast_ap)

        negG = wp.tile([128, 1], f32)
        nc.vector.memset(negG[:], -G)
        ones128_bf = wp.tile([128, 128], bf16)
        nc.vector.memset(ones128_bf[:], 1.0)

        # bias_out[p, e] = sum_a bv[a]*Wo[a, e] + bo[e]  (same for every p)
        ps_bo = pm.tile([128, E], f32, tag="pm")
        for at in range(AT):
            bv_rep = small.tile([128, 128], bf16, tag="bvrep")
            nc.vector.tensor_scalar_mul(bv_rep[:], ones128_bf[:],
                                        bv_sb[:, at:at + 1])
            nc.tensor.matmul(ps_bo[:], bv_rep[:], Wo_bf[:, at, :],
                             start=(at == 0), stop=(at == AT - 1))
        bias_out = wp.tile([128, E], f32)
        nc.vector.tensor_add(bias_out[:], ps_bo[:], boB[:])

        # ---- per-batch pipeline ----
        for b in range(BL):
            # A) stage + transpose inputs
            xq_st = stage.tile([128, QT, E], f32r, tag="stage")
            nc.gpsimd.dma_start(xq_st[:], lstm[b].rearrange("(t p) e -> p t e", p=128))
            XqT = big.tile([128, ET, LQ], f32r, tag="xqt")
            for t in range(QT):
                for es in range(ET):
                    p = pt.tile([128, 128], f32r, tag="pt")
                    nc.tensor.transpose(p[:], xq_st[:, t, es * 128:(es + 1) * 128],
                                        ident_r[:])
                    nc.vector.tensor_copy(XqT[:, es, t * 128:(t + 1) * 128], p[:])

            xk_st = stage.tile([128, KT, F], f32r, tag="stage")
            nc.gpsimd.dma_start(xk_st[:], flow[b].rearrange("(t p) f -> p t f", p=128))
            XkT = big.tile([128, FT, LK], f32r, tag="xkt")
            for t in range(KT):
                for fs in range(FT):
                    p = pt.tile([128, 128], f32r, tag="pt")
                    nc.tensor.transpose(p[:], xk_st[:, t, fs * 128:(fs + 1) * 128],
                                        ident_r[:])
                    nc.vector.tensor_copy(XkT[:, fs, t * 128:(t + 1) * 128], p[:])

            # B) projections
            qT = big.tile([128, AT, LQ], f32r, tag="qt")
            for at in range(AT):
                for qc in range(QC):
                    p = pm.tile([128, 512], f32, tag="pm")
                    for es in range(ET):
                        nc.tensor.matmul(
                            p[:], Wq_r[:, es, at * 128:(at + 1) * 128],
                            XqT[:, es, qc * 512:(qc + 1) * 512],
                            start=(es == 0), stop=(es == ET - 1))
                    nc.vector.tensor_scalar(
                        out=qT[:, at, qc * 512:(qc + 1) * 512], in0=p[:],
                        scalar1=bq_sb[:, at:at + 1], scalar2=None, op0=mybir.AluOpType.add)

            kT = big.tile([128, AT, LK], f32r, tag="kt")
            for at in range(AT):
                for kc in range(KC):
                    p = pm.tile([128, 512], f32, tag="pm")
                    for fs in range(FT):
                        nc.tensor.matmul(
                            p[:], Wk_r[:, fs, at * 128:(at + 1) * 128],
                            XkT[:, fs, kc * 512:(kc + 1) * 512],
                            start=(fs == 0), stop=(fs == FT - 1))
                    nc.vector.tensor_scalar(
                        out=kT[:, at, kc * 512:(kc + 1) * 512], in0=p[:],
                        scalar1=bk_sb[:, at:at + 1], scalar2=None, op0=mybir.AluOpType.add)

            v_bf = big.tile([128, KT, A], bf16, tag="v")
            for lt in range(KT):
                p = pm.tile([128, 512], f32, tag="pm")
                for fs in range(FT):
                    nc.tensor.matmul(
                        p[:], XkT[:, fs, lt * 128:(lt + 1) * 128], Wv_r[:, fs, :],
                        start=(fs == 0), stop=(fs == FT - 1))
                nc.vector.tensor_copy(v_bf[:, lt, :], p[:])

            # C) scoresT -> exp -> partial denominator
            expT = big.tile([128, KT, LQ], bf16, tag="expt")
            dacc = big.tile([128, LQ], f32, tag="dacc")
            for lt in range(KT):
                for qc in range(QC):
                    p = pm.tile([128, 512], f32, tag="pm")
                    for at in range(AT):
                        nc.tensor.matmul(
                            p[:], kT[:, at, lt * 128:(lt + 1) * 128],
                            qT[:, at, qc * 512:(qc + 1) * 512],
                            start=(at == 0), stop=(at == AT - 1))
                    nc.scalar.activation(
                        out=expT[:, lt, qc * 512:(qc + 1) * 512], in_=p[:],
                        func=mybir.ActivationFunctionType.Exp,
                        bias=negG[:], scale=1.0)
                if lt == 0:
                    nc.vector.tensor_copy(dacc[:], expT[:, 0, :])
                else:
                    nc.vector.tensor_add(dacc[:], dacc[:], expT[:, lt, :])

            # D[q] per q-tile via matmul rider: dacc^T @ ones
            dacc_bf = big.tile([128, LQ], bf16, tag="daccr")
            nc.vector.tensor_copy(dacc_bf[:], dacc[:])
            ps_d = pdp.tile([128, 8], f32, tag="pd")
            for qt in range(QT):
                nc.tensor.matmul(ps_d[:, qt:qt + 1],
                                 dacc_bf[:, qt * 128:(qt + 1) * 128],
                                 ones128_bf[:, 0:1],
                                 start=True, stop=True)
            recipD = small.tile([128, 8], f32, tag="recip")
            nc.vector.reciprocal(recipD[:], ps_d[:])

            # D) context (unnormalized, transposed) + final projection
            CT = big.tile([128, AT, LQ], bf16, tag="ct")
            for at in range(AT):
                for qc in range(QC):
                    p = pm.tile([128, 512], f32, tag="pm")
                    for lt in range(KT):
                        nc.tensor.matmul(
                            p[:], v_bf[:, lt, at * 128:(at + 1) * 128],
                            expT[:, lt, qc * 512:(qc + 1) * 512],
                            start=(lt == 0), stop=(lt == KT - 1))
                    nc.vector.tensor_copy(CT[:, at, qc * 512:(qc + 1) * 512], p[:])

            for qt in range(QT):
                p = pm.tile([128, 512], f32, tag="pm")
                for at in range(AT):
                    nc.tensor.matmul(
                        p[:], CT[:, at, qt * 128:(qt + 1) * 128], Wo_bf[:, at, :],
                        start=(at == 0), stop=(at == AT - 1))
                o_sb = small.tile([128, E], f32, tag="osb")
                nc.vector.tensor_scalar(
                    out=o_sb[:], in0=p[:], scalar1=recipD[:, qt:qt + 1], scalar2=None,
                    op0=mybir.AluOpType.mult)
                nc.vector.tensor_add(o_sb[:], o_sb[:], bias_out[:])
                nc.sync.dma_start(out_d[b, qt * 128:(qt + 1) * 128, :], o_sb[:])


_NC_CACHE = []


def _get_nc():
    if not _NC_CACHE:
        nc = bacc.Bacc("TRN2", target_bir_lowering=False, debug=False)
        with tile.TileContext(nc) as tc:
            _body(tc)
        nc.compile()
        _NC_CACHE.append(nc)
    return _NC_CACHE[0]


def kernel(trace=False, **inputs):
    lstm = np.ascontiguousarray(np.asarray(inputs["lstm_embeddings"], dtype=np.float32))
    flow = np.ascontiguousarray(np.asarray(inputs["optical_flow_features"], dtype=np.float32))
    base = {k: np.ascontiguousarray(np.asarray(inputs[k], dtype=np.float32))
            for k in ("Wq", "bq", "Wk", "bk", "Wv", "bv", "Wo", "bo")}
    base["ident_"] = np.eye(128, dtype=np.float32)
    base["ones_"] = np.ones((128, 1), dtype=np.float32)

    nc = _get_nc()
    in_maps = []
    for c in range(NCORES):
        m = dict(base)
        m["lstm_embeddings"] = lstm[c * BL:(c + 1) * BL]
        m["optical_flow_features"] = flow[c * BL:(c + 1) * BL]
        in_maps.append(m)

    kw = {}
    if trace:
        kw = dict(trace=True, trace_cores=[0])
    res = run_bass_kernel_spmd(nc, in_maps, core_ids=list(range(NCORES)), **kw)
    out = np.concatenate([r["out"] for r in res.results], axis=0)
    if trace:
        return out, res
    return out
